# revision 14
# baseline (speedup 1.0000x reference)
"""Trainium2 Bass kernel for CustomAttnDecoderRNN (B=64,S=128,E=512,H=1024,V=32000).

Strategy (8 NeuronCores):
  Phase A (Bahdanau attention): shard batch B across cores (8 rows each).
    energy restructured: tanh(enc @ Wa_e.T + (h @ Wa_h.T + ba)) -- halves FLOPs
    vs. the concat formulation.
  Phase B (custom GRU cell): shard the gate output dim H across cores
    (128 columns of r/u/n per core), full batch. Needs full `context`
    -> on-device AllGather #1 (8x[8,1024] -> [64,1024]).
  Phase C (vocab projection): shard V across cores (4000 rows of Wout each),
    full batch. Needs full h_new -> on-device AllGather #2
    (8x[128,64] h_newT slices -> [1024,64] = h_newT).
  Host reassembles: logits concat along V, h_new concat along H, attn
  weights concat along B.

All matmuls run as float32r (fp32 data, full-rate PE streaming mode).
"""

import numpy as np

from concourse import bass, bacc, mybir, tile
from concourse import bass_utils

B, S, E, H, V = 64, 128, 512, 1024, 32000
NC = 8            # cores
BL = B // NC      # local batch rows for attention        = 8
HL = H // NC      # local gate columns for GRU            = 128
VL = V // NC      # local vocab rows for projection       = 4000
G3 = 3 * HL       # per-core gate rows (r,u,n slices)     = 384
NT = 10           # vocab n-tiles per core
NTW = VL // NT    # vocab n-tile width                    = 400

F32 = mybir.dt.float32
F32R = mybir.dt.float32r
AX = mybir.AxisListType
AF = mybir.ActivationFunctionType


def _build_program():
    nc = bacc.Bacc(
        "TRN2", target_bir_lowering=False, debug=False, num_devices=NC
    )

    def din(name, shape, dt=F32):
        return nc.dram_tensor(name, list(shape), dt, kind="ExternalInput").ap()

    def dout(name, shape):
        return nc.dram_tensor(name, list(shape), F32, kind="ExternalOutput").ap()

    # ---- per-core external inputs ----
    encT = din("encT", (H, BL * S), F32R)      # [h, b*S+s] transposed encoder slice
    encN = din("encN", (S, BL * H), F32R)      # [s, b*H+h] natural encoder slice
    waTh = din("waTh", (H, H), F32R)           # Wa[:, :H].T
    waTe = din("waTe", (H, H), F32R)           # Wa[:, H:].T
    ba_d = din("ba", (1, H), F32R)
    v_d = din("v", (1, H))
    hlTloc = din("hlTloc", (H, BL), F32R)      # h_last[local rows].T
    embT = din("embT", (E, B), F32R)
    hlT = din("hlT", (H, B), F32R)
    zT = din("zT", (H, B), F32R)
    wxT = din("wxT", (E, G3), F32R)
    whT = din("whT", (H, G3), F32R)
    wcT = din("wcT", (H, G3), F32R)
    wzT = din("wzT", (H, G3), F32R)
    bxcz = din("bxcz", (1, G3), F32R)
    bh_d = din("bh", (1, G3), F32R)
    hlsl = din("hlsl", (B, HL))          # h_last[:, local cols]
    woutT = din("woutT", (H, VL), F32R)
    bout = din("bout", (1, VL), F32R)
    ones_d = din("ones", (1, 128), F32R)
    ident = din("ident", (128, 128))

    logits_o = dout("logits", (B, VL))
    hnewsl_o = dout("hnewsl", (B, HL))
    attnw_o = dout("attnw", (BL, S))

    rg = [list(range(NC))]

    with tile.TileContext(nc) as tc:
        with (
            tc.tile_pool(name="const", bufs=1) as cp,
            tc.tile_pool(name="dram", bufs=1, space="DRAM") as dp,
            tc.tile_pool(name="wout_pool", bufs=3) as wp,
            tc.tile_pool(name="phc", bufs=1) as pc,
        ):
            # ---------- constants ----------
            ident_sb = cp.tile([128, 128], F32)
            nc.sync.dma_start(ident_sb[:], ident[:, :])
            ones_sb = cp.tile([1, 128], F32R)
            nc.sync.dma_start(ones_sb[:], ones_d[:, :])
            vrep_sb = cp.tile([128, H], F32)
            nc.sync.dma_start(vrep_sb[:], v_d.partition_broadcast(128))
            ba_sb = cp.tile([1, H], F32R)
            nc.sync.dma_start(ba_sb[:], ba_d[:, :])
            bxcz_sb = cp.tile([1, G3], F32R)
            nc.sync.dma_start(bxcz_sb[:], bxcz[:, :])
            bh_sb = cp.tile([1, G3], F32R)
            nc.sync.dma_start(bh_sb[:], bh_d[:, :])
            bout_sb = cp.tile([1, VL], F32R)
            nc.sync.dma_start(bout_sb[:], bout[:, :])

            hterm_sb = cp.tile([BL, H], F32R)
            scores_sT = cp.tile([128, BL], F32)
            attnw_sb = cp.tile([BL, S], F32)
            attnT_sb = cp.tile([128, BL], F32R)

            # collective bounce buffers
            cc1_in = dp.tile([B, H], F32R, name="cc1_in")  # only BL rows used/core
            cc1_out = dp.tile([B, H], F32R, addr_space="Shared", name="cc1_out")
            cc2_in = dp.tile([HL, B], F32R, name="cc2_in")
            cc2_out = dp.tile([H, B], F32R, addr_space="Shared", name="cc2_out")

            # ---------- phase C weight stream (DMAs issued via scalar ring
            # so they fill DMA idle time without blocking sync-ring loads) ---
            # tiles created in the phase C loop below out of wp (bufs=3)

            # ---------- phase A0: hterm = h_last_loc @ Wa_h.T + ba ----------
            with (
                tc.tile_pool(name="pa0", bufs=1) as pa0,
                tc.tile_pool(name="psA0", bufs=1, space="PSUM") as psA0,
            ):
                hl_sb = pa0.tile([128, BL * 8], F32R)
                nc.sync.dma_start(
                    hl_sb[:],
                    hlTloc.rearrange("(kc p) b -> p kc b", p=128),
                )
                waTh_sb = pa0.tile([128, H * 8], F32R)
                nc.sync.dma_start(
                    waTh_sb[:],
                    waTh.rearrange("(kc p) j -> p kc j", p=128),
                )
                hterm_ps = psA0.tile([BL, H], F32)
                for jn in range(2):
                    o = hterm_ps[:, jn * 512:(jn + 1) * 512]
                    nc.tensor.matmul(
                        o, ones_sb[0:1, 0:BL],
                        ba_sb[0:1, jn * 512:(jn + 1) * 512],
                        start=True, stop=False,
                    )
                    for kc in range(8):
                        nc.tensor.matmul(
                            o,
                            hl_sb[:, kc * BL:(kc + 1) * BL],
                            waTh_sb[:, kc * H + jn * 512: kc * H + jn * 512 + 512],
                            start=False, stop=(kc == 7),
                        )
                nc.scalar.copy(hterm_sb[:], hterm_ps[:])

            # ---------- phase A1: energy, scores, softmax, context ----------
            with tc.tile_pool(name="pa1", bufs=1) as pa1:
                waTe_sb = pa1.tile([128, H * 8], F32R)
                nc.sync.dma_start(
                    waTe_sb[:], waTe.rearrange("(kc p) j -> p kc j", p=128)
                )
                encT_sb = pa1.tile([128, 8 * BL * S], F32R)
                nc.sync.dma_start(
                    encT_sb[:], encT.rearrange("(kc p) r -> p kc r", p=128)
                )
                encN_sb = pa1.tile([S, BL * H], F32R)
                nc.sync.dma_start(encN_sb[:], encN[:, :])

                with tc.tile_pool(name="psE", bufs=2, space="PSUM") as psE:
                    for b in range(BL):
                        eps = psE.tile([128, H], F32, tag="eps", name=f"eps{b}")
                        # stage hterm row b at partition 0 (matmul rhs base 0)
                        hrow = pa1.tile([1, H], F32R, tag="hrow", name=f"hrow{b}")
                        nc.sync.dma_start(hrow[:], hterm_sb[b:b + 1, :])
                        for jn in range(2):
                            o = eps[:, jn * 512:(jn + 1) * 512]
                            # broadcast hterm row b across 128 s-rows (rank-1)
                            nc.tensor.matmul(
                                o, ones_sb[0:1, :],
                                hrow[0:1, jn * 512:(jn + 1) * 512],
                                start=True, stop=False,
                            )
                            for kc in range(8):
                                nc.tensor.matmul(
                                    o,
                                    encT_sb[:, kc * 1024 + b * S: kc * 1024 + b * S + S],
                                    waTe_sb[:, kc * H + jn * 512: kc * H + jn * 512 + 512],
                                    start=False, stop=(kc == 7),
                                )
                        tanh_t = pa1.tile([128, H], F32, tag="tanh", name=f"tanh{b}")
                        nc.scalar.activation(tanh_t[:], eps[:], AF.Tanh)
                        scr = pa1.tile([128, H], F32, tag="scr", name=f"scr{b}")
                        nc.vector.tensor_tensor(
                            scr[:], tanh_t[:], vrep_sb[:], mybir.AluOpType.mult
                        )
                        nc.vector.reduce_sum(
                            scores_sT[:, b:b + 1], scr[:], axis=AX.X
                        )

                # softmax over s for each local batch row
                with tc.tile_pool(name="psS", bufs=1, space="PSUM") as psS:
                    scT_ps = psS.tile([BL, 128], F32)
                    nc.tensor.transpose(scT_ps[:], scores_sT[:], ident_sb[:])
                    negmax = cp.tile([BL, 1], F32)
                    nc.vector.reduce_max(
                        negmax[:], scT_ps[:], axis=AX.X, negate=True
                    )
                    expv = cp.tile([BL, S], F32)
                    nc.scalar.activation(
                        expv[:], scT_ps[:], AF.Exp, bias=negmax[:]
                    )
                    ssum = cp.tile([BL, 1], F32)
                    nc.vector.reduce_sum(ssum[:], expv[:], axis=AX.X)
                    rsum = cp.tile([BL, 1], F32)
                    nc.vector.reciprocal(rsum[:], ssum[:])
                    nc.vector.tensor_scalar_mul(attnw_sb[:], expv[:], rsum[:])
                    nc.sync.dma_start(attnw_o[:, :], attnw_sb[:])

                    attnT_ps = psS.tile([128, BL], F32)
                    nc.tensor.transpose(
                        attnT_ps[:], attnw_sb[:], ident_sb[0:BL, 0:BL]
                    )
                    nc.vector.tensor_copy(attnT_sb[:], attnT_ps[:])

                    # context rows -> cc1_in, then AllGather
                    for b in range(BL):
                        cps = psS.tile([1, H], F32, tag="cps", name=f"cps{b}")
                        for jn in range(2):
                            nc.tensor.matmul(
                                cps[:, jn * 512:(jn + 1) * 512],
                                attnT_sb[:, b:b + 1],
                                encN_sb[:, b * H + jn * 512: b * H + jn * 512 + 512],
                                start=True, stop=True,
                            )
                        csb = pa1.tile([1, H], F32R, tag="csb", name=f"csb{b}")
                        nc.scalar.copy(csb[:], cps[0:1, :])
                        nc.sync.dma_start(cc1_in[b:b + 1, :], csb[:])

            nc.gpsimd.collective_compute(
                "AllGather",
                mybir.AluOpType.bypass,
                replica_groups=rg,
                ins=[cc1_in[0:BL, :].bitcast(F32).opt()],
                outs=[cc1_out[:, :].bitcast(F32).opt()],
            )

            # ---------- phase B: custom GRU cell (H-sharded) ----------
            with (
                tc.tile_pool(name="pb", bufs=1) as pb,
                tc.tile_pool(name="psB", bufs=1, space="PSUM") as psB,
            ):
                wxT_sb = pb.tile([128, 4 * G3], F32R)
                nc.sync.dma_start(
                    wxT_sb[:], wxT.rearrange("(kc p) j -> p kc j", p=128)
                )
                whT_sb = pb.tile([128, 8 * G3], F32R)
                nc.sync.dma_start(
                    whT_sb[:], whT.rearrange("(kc p) j -> p kc j", p=128)
                )
                wcT_sb = pb.tile([128, 8 * G3], F32R)
                nc.sync.dma_start(
                    wcT_sb[:], wcT.rearrange("(kc p) j -> p kc j", p=128)
                )
                wzT_sb = pb.tile([128, 8 * G3], F32R)
                nc.sync.dma_start(
                    wzT_sb[:], wzT.rearrange("(kc p) j -> p kc j", p=128)
                )
                embT_sb = pb.tile([128, 4 * B], F32R)
                nc.sync.dma_start(
                    embT_sb[:], embT.rearrange("(kc p) b -> p kc b", p=128)
                )
                hlT_sb = pb.tile([128, 8 * B], F32R)
                nc.sync.dma_start(
                    hlT_sb[:], hlT.rearrange("(kc p) b -> p kc b", p=128)
                )
                zT_sb = pb.tile([128, 8 * B], F32R)
                nc.sync.dma_start(
                    zT_sb[:], zT.rearrange("(kc p) b -> p kc b", p=128)
                )
                hlsl_sb = pb.tile([B, HL], F32)
                nc.sync.dma_start(hlsl_sb[:], hlsl[:, :])
                ctxT_sb = pb.tile([128, 8 * B], F32R)
                for kc in range(8):
                    nc.sync.dma_start(
                        ctxT_sb[:, kc * B:(kc + 1) * B],
                        cc1_out[:, kc * 128:(kc + 1) * 128].transpose([1, 0]),
                    )

                gx_ps = psB.tile([B, G3], F32)
                nc.tensor.matmul(
                    gx_ps[:], ones_sb[0:1, 0:B], bxcz_sb[0:1, :],
                    start=True, stop=False,
                )
                for kc in range(4):
                    nc.tensor.matmul(
                        gx_ps[:], embT_sb[:, kc * B:(kc + 1) * B],
                        wxT_sb[:, kc * G3:(kc + 1) * G3],
                        start=False, stop=False,
                    )
                for kc in range(8):
                    nc.tensor.matmul(
                        gx_ps[:], ctxT_sb[:, kc * B:(kc + 1) * B],
                        wcT_sb[:, kc * G3:(kc + 1) * G3],
                        start=False, stop=False,
                    )
                for kc in range(8):
                    nc.tensor.matmul(
                        gx_ps[:], zT_sb[:, kc * B:(kc + 1) * B],
                        wzT_sb[:, kc * G3:(kc + 1) * G3],
                        start=False, stop=(kc == 7),
                    )
                gh_ps = psB.tile([B, G3], F32)
                nc.tensor.matmul(
                    gh_ps[:], ones_sb[0:1, 0:B], bh_sb[0:1, :],
                    start=True, stop=False,
                )
                for kc in range(8):
                    nc.tensor.matmul(
                        gh_ps[:], hlT_sb[:, kc * B:(kc + 1) * B],
                        whT_sb[:, kc * G3:(kc + 1) * G3],
                        start=False, stop=(kc == 7),
                    )
                gh_sb = pb.tile([B, G3], F32)
                nc.scalar.copy(gh_sb[:], gh_ps[:])

                xr = gx_ps[:, 0:HL]
                xu = gx_ps[:, HL:2 * HL]
                xn = gx_ps[:, 2 * HL:3 * HL]
                hr = gh_sb[:, 0:HL]
                hu = gh_sb[:, HL:2 * HL]
                hn = gh_sb[:, 2 * HL:3 * HL]

                t1 = pb.tile([B, HL], F32)
                nc.vector.tensor_tensor(t1[:], xr, hr, mybir.AluOpType.add)
                r_sb = pb.tile([B, HL], F32)
                nc.scalar.activation(r_sb[:], t1[:], AF.Sigmoid)
                t2 = pb.tile([B, HL], F32)
                nc.vector.tensor_tensor(t2[:], xu, hu, mybir.AluOpType.add)
                u_sb = pb.tile([B, HL], F32)
                nc.scalar.activation(u_sb[:], t2[:], AF.Sigmoid)
                t3 = pb.tile([B, HL], F32)
                nc.vector.tensor_tensor(t3[:], r_sb[:], hn, mybir.AluOpType.mult)
                t4 = pb.tile([B, HL], F32)
                nc.vector.tensor_tensor(t4[:], t3[:], xn, mybir.AluOpType.add)
                n_sb = pb.tile([B, HL], F32)
                nc.scalar.activation(n_sb[:], t4[:], AF.Tanh)
                # h_new = n + u * (h_last - n)
                t5 = pb.tile([B, HL], F32)
                nc.vector.tensor_tensor(
                    t5[:], hlsl_sb[:], n_sb[:], mybir.AluOpType.subtract
                )
                t6 = pb.tile([B, HL], F32)
                nc.vector.tensor_tensor(t6[:], u_sb[:], t5[:], mybir.AluOpType.mult)
                hnew_sb = pb.tile([B, HL], F32)
                nc.vector.tensor_tensor(
                    hnew_sb[:], n_sb[:], t6[:], mybir.AluOpType.add
                )
                nc.sync.dma_start(hnewsl_o[:, :], hnew_sb[:])

                hnT_ps = psB.tile([HL, B], F32)
                nc.tensor.transpose(
                    hnT_ps[:], hnew_sb[:], ident_sb[0:B, 0:B]
                )
                hnT_sb = pb.tile([HL, B], F32R)
                nc.vector.tensor_copy(hnT_sb[:], hnT_ps[:])
                nc.sync.dma_start(cc2_in[:, :], hnT_sb[:])

            nc.gpsimd.collective_compute(
                "AllGather",
                mybir.AluOpType.bypass,
                replica_groups=rg,
                ins=[cc2_in[:, :].bitcast(F32).opt()],
                outs=[cc2_out[:, :].bitcast(F32).opt()],
            )

            # ---------- phase C: vocab projection (V-sharded) ----------
            with tc.tile_pool(name="psC", bufs=2, space="PSUM") as psC:
                hnT_all = pc.tile([128, 8 * B], F32R)
                nc.sync.dma_start(
                    hnT_all[:], cc2_out.rearrange("(kc p) b -> p kc b", p=128)
                )
                for nt in range(NT):
                    w_sb = wp.tile(
                        [128, 8 * NTW], F32R, tag="wout", name=f"wout{nt}"
                    )
                    nc.scalar.dma_start(
                        w_sb[:],
                        woutT[:, nt * NTW:(nt + 1) * NTW].rearrange(
                            "(kc p) j -> p kc j", p=128
                        ),
                    )
                    lp = psC.tile([B, NTW], F32, tag="lp", name=f"lp{nt}")
                    nc.tensor.matmul(
                        lp[:], ones_sb[0:1, 0:B],
                        bout_sb[0:1, nt * NTW:(nt + 1) * NTW],
                        start=True, stop=False,
                    )
                    for kc in range(8):
                        nc.tensor.matmul(
                            lp[:], hnT_all[:, kc * B:(kc + 1) * B],
                            w_sb[:, kc * NTW:(kc + 1) * NTW],
                            start=False, stop=(kc == 7),
                        )
                    lsb = pc.tile(
                        [B, NTW], F32, tag="lsb", bufs=2, name=f"lsb{nt}"
                    )
                    nc.scalar.copy(lsb[:], lp[:])
                    nc.sync.dma_start(
                        logits_o[:, nt * NTW:(nt + 1) * NTW], lsb[:]
                    )

    nc.compile()
    return nc


def _prep_inputs(inputs):
    """Host-side sharding: build the per-core in_maps."""
    f = lambda a: np.ascontiguousarray(np.asarray(a), dtype=np.float32)
    emb = f(inputs["emb"])
    hidden = f(inputs["hidden"])
    enc = f(inputs["encoder_outputs"])
    z = f(inputs["z"])
    Wa, ba, v = f(inputs["Wa"]), f(inputs["ba"]), f(inputs["v"])
    Wx, bx = f(inputs["Wx"]), f(inputs["bx"])
    Wh, bh = f(inputs["Wh"]), f(inputs["bh"])
    Wc, bc = f(inputs["Wc"]), f(inputs["bc"])
    Wz, bz = f(inputs["Wz"]), f(inputs["bz"])
    Wout, bout = f(inputs["Wout"]), f(inputs["bout"])

    h_last = hidden[-1]                       # [B, H]
    c = np.ascontiguousarray

    waTh = c(Wa[:, :H].T)
    waTe = c(Wa[:, H:].T)
    ba_r = ba.reshape(1, H)
    v_r = v.reshape(1, H)
    embT = c(emb.T)
    hlT = c(h_last.T)
    zT = c(z.T)
    bxcz_full = bx + bc + bz
    ident = np.eye(128, dtype=np.float32)

    in_maps = []
    for ci in range(NC):
        bs = slice(ci * BL, (ci + 1) * BL)
        esl = enc[:, bs, :]                            # [S, BL, H]
        encT = c(esl.transpose(2, 1, 0)).reshape(H, BL * S)
        encN = c(esl).reshape(S, BL * H)
        rows = np.concatenate([
            np.arange(ci * HL, (ci + 1) * HL),
            np.arange(H + ci * HL, H + (ci + 1) * HL),
            np.arange(2 * H + ci * HL, 2 * H + (ci + 1) * HL),
        ])
        in_maps.append({
            "encT": encT,
            "encN": encN,
            "waTh": waTh,
            "waTe": waTe,
            "ba": ba_r,
            "v": v_r,
            "hlTloc": c(h_last[bs, :].T),
            "embT": embT,
            "hlT": hlT,
            "zT": zT,
            "wxT": c(Wx[rows, :].T),
            "whT": c(Wh[rows, :].T),
            "wcT": c(Wc[rows, :].T),
            "wzT": c(Wz[rows, :].T),
            "bxcz": c(bxcz_full[rows]).reshape(1, G3),
            "bh": c(bh[rows]).reshape(1, G3),
            "hlsl": c(h_last[:, ci * HL:(ci + 1) * HL]),
            "woutT": c(Wout[ci * VL:(ci + 1) * VL, :].T),
            "bout": c(bout[ci * VL:(ci + 1) * VL]).reshape(1, VL),
            "ones": np.ones((1, 128), np.float32),
            "ident": ident,
        })
    return in_maps


_PROGRAM = None


def _get_program():
    global _PROGRAM
    if _PROGRAM is None:
        _PROGRAM = _build_program()
    return _PROGRAM


def _assemble(results):
    logits = np.concatenate([r["logits"] for r in results], axis=1)
    h_new = np.concatenate([r["hnewsl"] for r in results], axis=1)[None]
    attnw = np.concatenate([r["attnw"] for r in results], axis=0)[:, None, :]
    return logits, h_new, attnw


def kernel(**inputs):
    nc = _get_program()
    in_maps = _prep_inputs(inputs)
    res = bass_utils.run_bass_kernel_spmd(
        nc, in_maps, core_ids=list(range(NC))
    )
    return _assemble(res.results)


# revision 18
# speedup vs baseline: 1.9375x; 1.9375x over previous
"""Trainium2 Bass kernel for CustomAttnDecoderRNN (B=64,S=128,E=512,H=1024,V=32000).

Strategy (8 NeuronCores):
  Phase A (Bahdanau attention): shard batch B across cores (8 rows each).
    energy restructured: tanh(enc @ Wa_e.T + (h @ Wa_h.T + ba)) -- halves FLOPs
    vs. the concat formulation.
  Phase B (custom GRU cell): shard the gate output dim H across cores
    (128 columns of r/u/n per core), full batch. Needs full `context`
    -> on-device AllGather #1 (8x[8,1024] -> [64,1024]).
  Phase C (vocab projection): shard V across cores (4000 rows of Wout each),
    full batch. Needs full h_new -> on-device AllGather #2
    (8x[128,64] h_newT slices -> [1024,64] = h_newT).
  Host reassembles: logits concat along V, h_new concat along H, attn
  weights concat along B.

All matmuls run as float32r (fp32 data, full-rate PE streaming mode).
"""

import numpy as np

from concourse import bass, bacc, mybir, tile
from concourse import bass_utils

B, S, E, H, V = 64, 128, 512, 1024, 32000
NC = 8            # cores
BL = B // NC      # local batch rows for attention        = 8
HL = H // NC      # local gate columns for GRU            = 128
VL = V // NC      # local vocab rows for projection       = 4000
G3 = 3 * HL       # per-core gate rows (r,u,n slices)     = 384
NT = 10           # vocab n-tiles per core
NTW = VL // NT    # vocab n-tile width                    = 400

F32 = mybir.dt.float32
F32R = mybir.dt.float32r
AX = mybir.AxisListType
AF = mybir.ActivationFunctionType


def _build_program():
    nc = bacc.Bacc(
        "TRN2", target_bir_lowering=False, debug=False, num_devices=NC
    )

    def din(name, shape, dt=F32):
        return nc.dram_tensor(name, list(shape), dt, kind="ExternalInput").ap()

    def dout(name, shape):
        return nc.dram_tensor(name, list(shape), F32, kind="ExternalOutput").ap()

    # ---- per-core external inputs ----
    encT = din("encT", (H, BL * S), F32R)      # [h, b*S+s] transposed encoder slice
    encN = din("encN", (S, BL * H), F32R)      # [s, b*H+h] natural encoder slice
    waTh = din("waTh", (H, H), F32R)           # Wa[:, :H].T
    waTe = din("waTe", (H, H), F32R)           # Wa[:, H:].T
    ba_d = din("ba", (1, H), F32R)
    v_d = din("v", (1, H))
    hlTloc = din("hlTloc", (H, BL), F32R)      # h_last[local rows].T
    embT = din("embT", (E, B), F32R)
    hlT = din("hlT", (H, B), F32R)
    zT = din("zT", (H, B), F32R)
    wxT = din("wxT", (E, G3), F32R)
    whT = din("whT", (H, G3), F32R)
    wcT = din("wcT", (H, G3), F32R)
    wzT = din("wzT", (H, G3), F32R)
    bxcz = din("bxcz", (1, G3), F32R)
    bh_d = din("bh", (1, G3), F32R)
    hlsl = din("hlsl", (B, HL))          # h_last[:, local cols]
    woutT = din("woutT", (H, VL), F32R)
    ones_d = din("ones", (1, 128), F32R)
    ident = din("ident", (128, 128))

    logits_o = dout("logits", (B, VL))
    hnewsl_o = dout("hnewsl", (B, HL))
    attnw_o = dout("attnw", (BL, S))

    rg = [list(range(NC))]

    with tile.TileContext(nc) as tc:
        with (
            tc.tile_pool(name="const", bufs=1) as cp,
            tc.tile_pool(name="dram", bufs=1, space="DRAM") as dp,
            tc.tile_pool(name="wout_pool", bufs=5) as wp,
            tc.tile_pool(name="phc", bufs=1) as pc,
        ):
            # ---------- constants ----------
            ident_sb = cp.tile([128, 128], F32)
            nc.sync.dma_start(ident_sb[:], ident[:, :])
            ones_sb = cp.tile([1, 128], F32R)
            nc.sync.dma_start(ones_sb[:], ones_d[:, :])
            vrep_sb = cp.tile([128, H], F32)
            nc.sync.dma_start(vrep_sb[:], v_d.partition_broadcast(128))
            ba_sb = cp.tile([1, H], F32R)
            nc.sync.dma_start(ba_sb[:], ba_d[:, :])
            bxcz_sb = cp.tile([1, G3], F32R)
            nc.sync.dma_start(bxcz_sb[:], bxcz[:, :])
            bh_sb = cp.tile([1, G3], F32R)
            nc.sync.dma_start(bh_sb[:], bh_d[:, :])

            hterm_sb = cp.tile([BL, H], F32R)
            scores_sT = cp.tile([128, BL], F32)
            attnw_sb = cp.tile([BL, S], F32)
            attnT_sb = cp.tile([128, BL], F32R)

            # collective bounce buffers
            cc1_in = dp.tile([B, H], F32R, name="cc1_in")  # only BL rows used/core
            cc1_out = dp.tile([B, H], F32R, addr_space="Shared", name="cc1_out")
            cc2_in = dp.tile([HL, B], F32R, name="cc2_in")
            cc2_out = dp.tile([H, B], F32R, addr_space="Shared", name="cc2_out")

            # ---------- phase C weight stream: issue the first 8 chunk DMAs
            # up front on the scalar ring so Wout flows during phases A/B ----
            wout_tiles = []
            for nt in range(5):
                w_sb = wp.tile(
                    [128, 8 * NTW], F32R, tag="wout", name=f"wout{nt}"
                )
                wout_tiles.append(w_sb)
                nc.scalar.dma_start(
                    w_sb[:],
                    woutT[:, nt * NTW:(nt + 1) * NTW].rearrange(
                        "(kc p) j -> p kc j", p=128
                    ),
                )

            # ---------- phase A0: hterm = h_last_loc @ Wa_h.T + ba ----------
            with (
                tc.tile_pool(name="pa0", bufs=1) as pa0,
                tc.tile_pool(name="psA0", bufs=1, space="PSUM") as psA0,
            ):
                hl_sb = pa0.tile([128, BL * 8], F32R)
                nc.sync.dma_start(
                    hl_sb[:],
                    hlTloc.rearrange("(kc p) b -> p kc b", p=128),
                )
                waTh_sb = pa0.tile([128, H * 8], F32R)
                nc.sync.dma_start(
                    waTh_sb[:],
                    waTh.rearrange("(kc p) j -> p kc j", p=128),
                )
                hterm_ps = psA0.tile([BL, H], F32)
                for jn in range(2):
                    o = hterm_ps[:, jn * 512:(jn + 1) * 512]
                    nc.tensor.matmul(
                        o, ones_sb[0:1, 0:BL],
                        ba_sb[0:1, jn * 512:(jn + 1) * 512],
                        start=True, stop=False,
                    )
                    for kc in range(8):
                        nc.tensor.matmul(
                            o,
                            hl_sb[:, kc * BL:(kc + 1) * BL],
                            waTh_sb[:, kc * H + jn * 512: kc * H + jn * 512 + 512],
                            start=False, stop=(kc == 7),
                        )
                nc.scalar.copy(hterm_sb[:], hterm_ps[:])

            # ---------- phase A1: energy, scores, softmax, context ----------
            with tc.tile_pool(name="pa1", bufs=1) as pa1:
                waTe_sb = pa1.tile([128, H * 8], F32R)
                nc.sync.dma_start(
                    waTe_sb[:], waTe.rearrange("(kc p) j -> p kc j", p=128)
                )
                encT_sb = pa1.tile([128, 8 * BL * S], F32R)
                nc.sync.dma_start(
                    encT_sb[:], encT.rearrange("(kc p) r -> p kc r", p=128)
                )
                encN_sb = pa1.tile([S, BL * H], F32R)
                nc.sync.dma_start(encN_sb[:], encN[:, :])

                with tc.tile_pool(name="psE", bufs=2, space="PSUM") as psE:
                    for b in range(BL):
                        eps = psE.tile([128, H], F32, tag="eps", name=f"eps{b}")
                        # stage hterm row b at partition 0 (matmul rhs base 0)
                        hrow = pa1.tile([1, H], F32R, tag="hrow", bufs=2, name=f"hrow{b}")
                        nc.sync.dma_start(hrow[:], hterm_sb[b:b + 1, :])
                        for jn in range(2):
                            o = eps[:, jn * 512:(jn + 1) * 512]
                            # broadcast hterm row b across 128 s-rows (rank-1)
                            nc.tensor.matmul(
                                o, ones_sb[0:1, :],
                                hrow[0:1, jn * 512:(jn + 1) * 512],
                                start=True, stop=False,
                            )
                            for kc in range(8):
                                nc.tensor.matmul(
                                    o,
                                    encT_sb[:, kc * 1024 + b * S: kc * 1024 + b * S + S],
                                    waTe_sb[:, kc * H + jn * 512: kc * H + jn * 512 + 512],
                                    start=False, stop=(kc == 7),
                                )
                        tanh_t = pa1.tile([128, H], F32, tag="tanh", name=f"tanh{b}")
                        nc.scalar.activation(tanh_t[:], eps[:], AF.Tanh)
                        scr = pa1.tile([128, H], F32, tag="scr", name=f"scr{b}")
                        nc.vector.tensor_tensor(
                            scr[:], tanh_t[:], vrep_sb[:], mybir.AluOpType.mult
                        )
                        nc.vector.reduce_sum(
                            scores_sT[:, b:b + 1], scr[:], axis=AX.X
                        )

                # softmax over s for each local batch row
                with tc.tile_pool(name="psS", bufs=1, space="PSUM") as psS:
                    scT_ps = psS.tile([BL, 128], F32)
                    nc.tensor.transpose(scT_ps[:], scores_sT[:], ident_sb[:])
                    negmax = cp.tile([BL, 1], F32)
                    nc.vector.reduce_max(
                        negmax[:], scT_ps[:], axis=AX.X, negate=True
                    )
                    expv = cp.tile([BL, S], F32)
                    nc.scalar.activation(
                        expv[:], scT_ps[:], AF.Exp, bias=negmax[:]
                    )
                    ssum = cp.tile([BL, 1], F32)
                    nc.vector.reduce_sum(ssum[:], expv[:], axis=AX.X)
                    rsum = cp.tile([BL, 1], F32)
                    nc.vector.reciprocal(rsum[:], ssum[:])
                    nc.vector.tensor_scalar_mul(attnw_sb[:], expv[:], rsum[:])
                    nc.sync.dma_start(attnw_o[:, :], attnw_sb[:])

                    attnT_ps = psS.tile([128, BL], F32)
                    nc.tensor.transpose(
                        attnT_ps[:], attnw_sb[:], ident_sb[0:BL, 0:BL]
                    )
                    nc.vector.tensor_copy(attnT_sb[:], attnT_ps[:])

                    # context rows -> cc1_in, then AllGather
                    for b in range(BL):
                        cps = psS.tile([1, H], F32, tag="cps", name=f"cps{b}")
                        for jn in range(2):
                            nc.tensor.matmul(
                                cps[:, jn * 512:(jn + 1) * 512],
                                attnT_sb[:, b:b + 1],
                                encN_sb[:, b * H + jn * 512: b * H + jn * 512 + 512],
                                start=True, stop=True,
                            )
                        csb = pa1.tile([1, H], F32R, tag="csb", name=f"csb{b}")
                        nc.scalar.copy(csb[:], cps[0:1, :])
                        nc.sync.dma_start(cc1_in[b:b + 1, :], csb[:])

            nc.gpsimd.collective_compute(
                "AllGather",
                mybir.AluOpType.bypass,
                replica_groups=rg,
                ins=[cc1_in[0:BL, :].bitcast(F32).opt()],
                outs=[cc1_out[:, :].bitcast(F32).opt()],
            )

            # ---------- remaining Wout chunks into the space pa1 freed ----
            wp2_ctx = tc.tile_pool(name="wout_pool2", bufs=5)
            wp2 = wp2_ctx.__enter__()
            for nt in range(5, NT):
                w_sb = wp2.tile(
                    [128, 8 * NTW], F32R, tag="wout2", name=f"wout{nt}"
                )
                wout_tiles.append(w_sb)
                nc.scalar.dma_start(
                    w_sb[:],
                    woutT[:, nt * NTW:(nt + 1) * NTW].rearrange(
                        "(kc p) j -> p kc j", p=128
                    ),
                )

            # ---------- phase B: custom GRU cell (H-sharded) ----------
            with (
                tc.tile_pool(name="pb", bufs=1) as pb,
                tc.tile_pool(name="psB", bufs=1, space="PSUM") as psB,
            ):
                wxT_sb = pb.tile([128, 4 * G3], F32R)
                nc.sync.dma_start(
                    wxT_sb[:], wxT.rearrange("(kc p) j -> p kc j", p=128)
                )
                whT_sb = pb.tile([128, 8 * G3], F32R)
                nc.sync.dma_start(
                    whT_sb[:], whT.rearrange("(kc p) j -> p kc j", p=128)
                )
                wcT_sb = pb.tile([128, 8 * G3], F32R)
                nc.sync.dma_start(
                    wcT_sb[:], wcT.rearrange("(kc p) j -> p kc j", p=128)
                )
                wzT_sb = pb.tile([128, 8 * G3], F32R)
                nc.sync.dma_start(
                    wzT_sb[:], wzT.rearrange("(kc p) j -> p kc j", p=128)
                )
                embT_sb = pb.tile([128, 4 * B], F32R)
                nc.sync.dma_start(
                    embT_sb[:], embT.rearrange("(kc p) b -> p kc b", p=128)
                )
                hlT_sb = pb.tile([128, 8 * B], F32R)
                nc.sync.dma_start(
                    hlT_sb[:], hlT.rearrange("(kc p) b -> p kc b", p=128)
                )
                zT_sb = pb.tile([128, 8 * B], F32R)
                nc.sync.dma_start(
                    zT_sb[:], zT.rearrange("(kc p) b -> p kc b", p=128)
                )
                hlsl_sb = pb.tile([B, HL], F32)
                nc.sync.dma_start(hlsl_sb[:], hlsl[:, :])
                ctx_nat = pb.tile([B, H], F32)
                nc.sync.dma_start(ctx_nat[:], cc1_out[:, :].bitcast(F32))
                ctxT_sb = pb.tile([128, 8 * B], F32R)
                for kc in range(8):
                    ctp = psB.tile([HL, B], F32, tag="ctp", bufs=2, name=f"ctp{kc}")
                    nc.tensor.transpose(
                        ctp[:], ctx_nat[:, kc * 128:(kc + 1) * 128],
                        ident_sb[0:B, 0:B],
                    )
                    nc.vector.tensor_copy(
                        ctxT_sb[:, kc * B:(kc + 1) * B], ctp[:]
                    )

                gx_ps = psB.tile([B, G3], F32)
                nc.tensor.matmul(
                    gx_ps[:], ones_sb[0:1, 0:B], bxcz_sb[0:1, :],
                    start=True, stop=False,
                )
                for kc in range(4):
                    nc.tensor.matmul(
                        gx_ps[:], embT_sb[:, kc * B:(kc + 1) * B],
                        wxT_sb[:, kc * G3:(kc + 1) * G3],
                        start=False, stop=False,
                    )
                for kc in range(8):
                    nc.tensor.matmul(
                        gx_ps[:], ctxT_sb[:, kc * B:(kc + 1) * B],
                        wcT_sb[:, kc * G3:(kc + 1) * G3],
                        start=False, stop=False,
                    )
                for kc in range(8):
                    nc.tensor.matmul(
                        gx_ps[:], zT_sb[:, kc * B:(kc + 1) * B],
                        wzT_sb[:, kc * G3:(kc + 1) * G3],
                        start=False, stop=(kc == 7),
                    )
                gh_ps = psB.tile([B, G3], F32)
                nc.tensor.matmul(
                    gh_ps[:], ones_sb[0:1, 0:B], bh_sb[0:1, :],
                    start=True, stop=False,
                )
                for kc in range(8):
                    nc.tensor.matmul(
                        gh_ps[:], hlT_sb[:, kc * B:(kc + 1) * B],
                        whT_sb[:, kc * G3:(kc + 1) * G3],
                        start=False, stop=(kc == 7),
                    )
                gh_sb = pb.tile([B, G3], F32)
                nc.scalar.copy(gh_sb[:], gh_ps[:])

                xr = gx_ps[:, 0:HL]
                xu = gx_ps[:, HL:2 * HL]
                xn = gx_ps[:, 2 * HL:3 * HL]
                hr = gh_sb[:, 0:HL]
                hu = gh_sb[:, HL:2 * HL]
                hn = gh_sb[:, 2 * HL:3 * HL]

                t1 = pb.tile([B, HL], F32)
                nc.vector.tensor_tensor(t1[:], xr, hr, mybir.AluOpType.add)
                r_sb = pb.tile([B, HL], F32)
                nc.scalar.activation(r_sb[:], t1[:], AF.Sigmoid)
                t2 = pb.tile([B, HL], F32)
                nc.vector.tensor_tensor(t2[:], xu, hu, mybir.AluOpType.add)
                u_sb = pb.tile([B, HL], F32)
                nc.scalar.activation(u_sb[:], t2[:], AF.Sigmoid)
                t3 = pb.tile([B, HL], F32)
                nc.vector.tensor_tensor(t3[:], r_sb[:], hn, mybir.AluOpType.mult)
                t4 = pb.tile([B, HL], F32)
                nc.vector.tensor_tensor(t4[:], t3[:], xn, mybir.AluOpType.add)
                n_sb = pb.tile([B, HL], F32)
                nc.scalar.activation(n_sb[:], t4[:], AF.Tanh)
                # h_new = n + u * (h_last - n)
                t5 = pb.tile([B, HL], F32)
                nc.vector.tensor_tensor(
                    t5[:], hlsl_sb[:], n_sb[:], mybir.AluOpType.subtract
                )
                t6 = pb.tile([B, HL], F32)
                nc.vector.tensor_tensor(t6[:], u_sb[:], t5[:], mybir.AluOpType.mult)
                hnew_sb = pb.tile([B, HL], F32)
                nc.vector.tensor_tensor(
                    hnew_sb[:], n_sb[:], t6[:], mybir.AluOpType.add
                )
                nc.sync.dma_start(hnewsl_o[:, :], hnew_sb[:])

                hnT_ps = psB.tile([HL, B], F32)
                nc.tensor.transpose(
                    hnT_ps[:], hnew_sb[:], ident_sb[0:B, 0:B]
                )
                hnT_sb = pb.tile([HL, B], F32R)
                nc.vector.tensor_copy(hnT_sb[:], hnT_ps[:])
                nc.sync.dma_start(cc2_in[:, :], hnT_sb[:])

            nc.gpsimd.collective_compute(
                "AllGather",
                mybir.AluOpType.bypass,
                replica_groups=rg,
                ins=[cc2_in[:, :].bitcast(F32).opt()],
                outs=[cc2_out[:, :].bitcast(F32).opt()],
            )

            # ---------- phase C: vocab projection (V-sharded) ----------
            with tc.tile_pool(name="psC", bufs=2, space="PSUM") as psC:
                hnT_all = pc.tile([128, 8 * B], F32R)
                nc.sync.dma_start(
                    hnT_all[:], cc2_out.rearrange("(kc p) b -> p kc b", p=128)
                )
                for nt in range(NT):
                    w_sb = wout_tiles[nt]
                    lp = psC.tile([B, NTW], F32, tag="lp", name=f"lp{nt}")
                    for kc in range(8):
                        nc.tensor.matmul(
                            lp[:], hnT_all[:, kc * B:(kc + 1) * B],
                            w_sb[:, kc * NTW:(kc + 1) * NTW],
                            start=(kc == 0), stop=(kc == 7),
                        )
                    lsb = pc.tile(
                        [B, NTW], F32, tag="lsb", bufs=2, name=f"lsb{nt}"
                    )
                    nc.vector.tensor_copy(lsb[:], lp[:])
                    nc.sync.dma_start(
                        logits_o[:, nt * NTW:(nt + 1) * NTW], lsb[:]
                    )
            wp2_ctx.__exit__(None, None, None)

    nc.compile()
    return nc


def _prep_inputs(inputs):
    """Host-side sharding: build the per-core in_maps."""
    f = lambda a: np.ascontiguousarray(np.asarray(a), dtype=np.float32)
    emb = f(inputs["emb"])
    hidden = f(inputs["hidden"])
    enc = f(inputs["encoder_outputs"])
    z = f(inputs["z"])
    Wa, ba, v = f(inputs["Wa"]), f(inputs["ba"]), f(inputs["v"])
    Wx, bx = f(inputs["Wx"]), f(inputs["bx"])
    Wh, bh = f(inputs["Wh"]), f(inputs["bh"])
    Wc, bc = f(inputs["Wc"]), f(inputs["bc"])
    Wz, bz = f(inputs["Wz"]), f(inputs["bz"])
    Wout, bout = f(inputs["Wout"]), f(inputs["bout"])

    h_last = hidden[-1]                       # [B, H]
    c = np.ascontiguousarray

    waTh = c(Wa[:, :H].T)
    waTe = c(Wa[:, H:].T)
    ba_r = ba.reshape(1, H)
    v_r = v.reshape(1, H)
    embT = c(emb.T)
    hlT = c(h_last.T)
    zT = c(z.T)
    bxcz_full = bx + bc + bz
    ident = np.eye(128, dtype=np.float32)

    in_maps = []
    for ci in range(NC):
        bs = slice(ci * BL, (ci + 1) * BL)
        esl = enc[:, bs, :]                            # [S, BL, H]
        encT = c(esl.transpose(2, 1, 0)).reshape(H, BL * S)
        encN = c(esl).reshape(S, BL * H)
        rows = np.concatenate([
            np.arange(ci * HL, (ci + 1) * HL),
            np.arange(H + ci * HL, H + (ci + 1) * HL),
            np.arange(2 * H + ci * HL, 2 * H + (ci + 1) * HL),
        ])
        in_maps.append({
            "encT": encT,
            "encN": encN,
            "waTh": waTh,
            "waTe": waTe,
            "ba": ba_r,
            "v": v_r,
            "hlTloc": c(h_last[bs, :].T),
            "embT": embT,
            "hlT": hlT,
            "zT": zT,
            "wxT": c(Wx[rows, :].T),
            "whT": c(Wh[rows, :].T),
            "wcT": c(Wc[rows, :].T),
            "wzT": c(Wz[rows, :].T),
            "bxcz": c(bxcz_full[rows]).reshape(1, G3),
            "bh": c(bh[rows]).reshape(1, G3),
            "hlsl": c(h_last[:, ci * HL:(ci + 1) * HL]),
            "woutT": c(Wout[ci * VL:(ci + 1) * VL, :].T),
            "ones": np.ones((1, 128), np.float32),
            "ident": ident,
        })
    return in_maps


_PROGRAM = None


def _get_program():
    global _PROGRAM
    if _PROGRAM is None:
        _PROGRAM = _build_program()
    return _PROGRAM


def _assemble(results, bout):
    logits = np.concatenate([r["logits"] for r in results], axis=1) + bout
    h_new = np.concatenate([r["hnewsl"] for r in results], axis=1)[None]
    attnw = np.concatenate([r["attnw"] for r in results], axis=0)[:, None, :]
    return logits, h_new, attnw


def kernel(**inputs):
    nc = _get_program()
    in_maps = _prep_inputs(inputs)
    res = bass_utils.run_bass_kernel_spmd(
        nc, in_maps, core_ids=list(range(NC))
    )
    return _assemble(res.results, np.asarray(inputs["bout"], np.float32))
